# revision 16
# baseline (speedup 1.0000x reference)
"""Trainium2 Bass kernel for chunked recurrent causal linear attention.

Problem: b=2, h=8, n=2048, d=128, e=64, chunk=128, two branches (plain +
rotary) sharing one denominator.

Math (per (b,h), per chunk c, token t in chunk, with running state
S[d,e], Z[d] per branch):
    AT[s,t]   = k_s . q_t                  (s,t in chunk; masked to s<=t)
    num[t,:]  = sum_s ATm[s,t] v_s + q_t @ S      (both branches summed)
    den[t]    = sum_s ATm[s,t]   + q_t . Z        (both branches summed)
    out[t,:]  = num[t,:] / den[t]
    S += k_chunk^T v_chunk ;  Z += sum_s k_s

Sharding: 16 (b,h) pairs over 8 cores, 2 pairs per core.

Implementation notes (v2):
  - All inputs in fp16: 2x less DMA traffic and 4x PE matmul throughput
    vs fp32 (fp32 matmuls lower to 2 half-speed passes). PSUM accumulation
    stays fp32. Measured end-to-end rel err 4.3e-4 vs the 2e-2 gate.
  - Host packs every per-chunk operand (qT/kT/qrT/krT pre-transposed,
    kn/krn natural for the state update, v plus a ones column) for both
    pairs into one [128, GW] group per CG chunks, so each input DMA is a
    single contiguous ~860KB transfer (~78% of peak vs ~30% for the old
    per-tensor 65-128KB transfers).
  - Output is written in SBUF-native layout [token-in-chunk, chunk, e]
    (contiguous 1KB-per-partition runs; fp16 rows in token-major order
    would be 128B runs, below the 512B DMA line-rate minimum) and
    inverse-permuted on host.
  - Both pairs share single PSUM banks for AT, num/den, and state, so the
    causal mask, state evacuation, and reciprocal run as ONE wide op per
    chunk instead of one per pair (halves DVE/ACT instruction count).
"""

import contextlib
import sys

_nullctx = contextlib.nullcontext

if "/opt/trn_rl_repo" not in sys.path:
    sys.path.insert(0, "/opt/trn_rl_repo")

import numpy as np

import concourse.bass as bass
import concourse.tile as tile
from concourse import bacc, mybir
from concourse.bass_utils import run_bass_kernel_spmd

F32 = mybir.dt.float32
F16 = mybir.dt.float16

N_CORES = 8
NP = 2             # (b,h) pairs per core
N = 2048           # sequence length per (b,h)
D = 128            # qk head dim
E = 64             # v head dim
E1 = E + 1         # v plus ones column
C = 128            # chunk size
NCHUNK = N // C    # 16

# input group packing: CG chunks x both pairs per DMA
CG = 2                      # chunks per group (per pair)
NG = NCHUNK // CG           # 8 groups
CW = 840                    # padded cols per (pair, chunk) section (16B align)
OFF_QT, OFF_KT, OFF_QRT, OFF_KRT = 0, 128, 256, 384
OFF_KN, OFF_KRN, OFF_V1 = 512, 640, 768
GW = NP * CG * CW           # 3360 cols = 6720B/partition per group

SW = 66            # state-bank region stride per (pair, branch) (>= E1)
PW = 72            # pout-bank region stride per pair (>= E1)
OSL = 8            # chunks per output slab
NOS = NCHUNK // OSL

_cached = {}


def build_kernel(repeat=1, loop_k=None, gbufs=8, dma_only=False,
                 compute_only=False):
    if compute_only:
        gbufs = max(gbufs, NG)
    nc = bacc.Bacc("TRN2", target_bir_lowering=False, debug=False,
                   num_devices=N_CORES)

    in_all = nc.dram_tensor("in_all", [NG * C, GW], F16,
                            kind="ExternalInput").ap()
    mask2 = nc.dram_tensor("mask2", [C, 2 * C], F32,
                           kind="ExternalInput").ap()
    out = nc.dram_tensor("out", [NP * NOS * C, OSL * E], F16,
                         kind="ExternalOutput").ap()

    with tile.TileContext(nc) as tc:
        with (
            tc.tile_pool(name="const", bufs=1) as constp,
            tc.tile_pool(name="grp", bufs=gbufs) as grpp,
            tc.tile_pool(name="atm", bufs=3) as atmp,
            tc.tile_pool(name="ssb", bufs=4) as ssbp,
            tc.tile_pool(name="dinv", bufs=8) as dinvp,
            tc.tile_pool(name="outs", bufs=2 * NP) as outsp,
            tc.tile_pool(name="pat", bufs=3, space="PSUM") as patp,
            tc.tile_pool(name="pout", bufs=3, space="PSUM") as poutp,
            tc.tile_pool(name="pst", bufs=1, space="PSUM") as pstp,
        ):
            mask_t = constp.tile([C, 2 * C], F32, tag="mask")
            nc.sync.dma_start(mask_t[:], mask2[:])

            for rep in range(repeat):
              # compute_only probe: load every group once, outside the
              # timed loop, so the loop body is pure engine work
              pre_tiles = {}
              if compute_only:
                  for g in range(NG):
                      gtile = grpp.tile([C, GW], F16, tag="grp",
                                        name=f"pg_{rep}_{g}")
                      nc.sync.dma_start(gtile[:], in_all[g * C:(g + 1) * C, :])
                      pre_tiles[g] = gtile
              with (tc.For_i(0, loop_k, 1, hint_engines=(
                        mybir.EngineType.PE, mybir.EngineType.DVE,
                        mybir.EngineType.Activation, mybir.EngineType.SP))
                    if (loop_k is not None and loop_k > 1)
                    else _nullctx()):
                # one state bank: region (p, br) at cols (2p+br)*SW
                pst = pstp.tile([D, 2 * NP * SW], F32, tag="pS",
                                name=f"pS_{rep}")

                group_tiles = {}
                S_box = [None]        # current [D, 4*SW] fp16 sbuf state
                outs_t = {}           # pair -> current output slab tile

                # Software pipeline, one chunk deep: the front stage of
                # chunk c emits the group load (every CG chunks), the state
                # update (PE), and AT+mask (PE then DVE); the back stage
                # consumes chunk c-1's masked AT for the num/den matmuls,
                # reciprocal and output scale. Every cross-engine hop gets
                # a full stage of slack.
                pending = None
                for cc in range(NCHUNK + 1):
                    back = pending
                    pending = None
                    if cc < NCHUNK:
                        c = cc
                        g, j = divmod(c, CG)
                        if compute_only:
                            group_tiles[g] = pre_tiles[g]
                        elif j == 0:
                            gtile = grpp.tile([C, GW], F16, tag="grp",
                                              name=f"g_{rep}_{g}")
                            nc.sync.dma_start(gtile[:],
                                              in_all[g * C:(g + 1) * C, :])
                            group_tiles[g] = gtile
                        gtile = group_tiles[g]

                        def sec(p, off, w, _j=j, _g=gtile):
                            b = (p * CG + _j) * CW + off
                            return _g[:, b:b + w]

                        sl = {}
                        for p in range(NP):
                            sl[p] = dict(
                                qcT=sec(p, OFF_QT, C),
                                kcT=sec(p, OFF_KT, C),
                                qrcT=sec(p, OFF_QRT, C),
                                krcT=sec(p, OFF_KRT, C),
                                knc=sec(p, OFF_KN, D),
                                krnc=sec(p, OFF_KRN, D),
                                vc=sec(p, OFF_V1, E1),
                            )
                        if dma_only:
                            # DMA floor probe: input stream only (outs tiles
                            # are never written, so shipping them is invalid)
                            continue

                        if c % OSL == 0:
                            for p in range(NP):
                                outs_t[p] = outsp.tile(
                                    [C, OSL * E], F16, tag="outs",
                                    name=f"o_{rep}_{p}_{c}")

                        prev_S = S_box[0]

                        # state update: all four (pair, branch) regions in
                        # one PSUM bank, accumulated across chunks
                        for br in range(2):
                            for p in range(NP):
                                z = sl[p]
                                kin = z["knc"] if br == 0 else z["krnc"]
                                nc.tensor.matmul(
                                    pst[:, (2 * p + br) * SW:
                                        (2 * p + br) * SW + E1],
                                    kin, z["vc"],
                                    start=(c == 0 and br == 0 and p == 0),
                                    stop=(c == NCHUNK - 1 and br == 1
                                          and p == NP - 1),
                                    skip_group_check=True)
                        if c < NCHUNK - 1:
                            s01 = ssbp.tile([D, 2 * NP * SW], F16, tag="ssb",
                                            name=f"s_{rep}_{c}")
                            nc.scalar.copy(s01[:], pst[:])
                            S_box[0] = s01

                        # AT for both pairs/branches into one bank, one mask
                        patb = patp.tile([C, 2 * C], F32, tag="pat",
                                         name=f"pat_{rep}_{c}")
                        for br in range(2):
                            for p in range(NP):
                                z = sl[p]
                                kk = z["kcT"] if br == 0 else z["krcT"]
                                qq = z["qcT"] if br == 0 else z["qrcT"]
                                nc.tensor.matmul(
                                    patb[:, p * C:(p + 1) * C], kk, qq,
                                    start=(br == 0 and p == 0),
                                    stop=(br == 1 and p == NP - 1),
                                    skip_group_check=True)
                        atm = atmp.tile([C, 2 * C], F16, tag="atm",
                                        name=f"atm_{rep}_{c}")
                        nc.vector.tensor_mul(atm[:], patb[:], mask_t[:])

                        pending = dict(atm=atm, sl=sl, c=c, prev_S=prev_S,
                                       outs=dict(outs_t))

                    if back is not None:
                        cb = back["c"]
                        pob = poutp.tile([C, NP * PW], F32, tag="po",
                                         name=f"po_{rep}_{cb}")
                        first = back["prev_S"] is None
                        for p in range(NP):
                            z = back["sl"][p]
                            nc.tensor.matmul(
                                pob[:, p * PW:p * PW + E1],
                                back["atm"][:, p * C:(p + 1) * C], z["vc"],
                                start=(p == 0),
                                stop=(first and p == NP - 1),
                                skip_group_check=True)
                        if not first:
                            pv = back["prev_S"]
                            for br in range(2):
                                for p in range(NP):
                                    z = back["sl"][p]
                                    qq = z["qcT"] if br == 0 else z["qrcT"]
                                    nc.tensor.matmul(
                                        pob[:, p * PW:p * PW + E1], qq,
                                        pv[:, (2 * p + br) * SW:
                                           (2 * p + br) * SW + E1],
                                        start=False,
                                        stop=(br == 1 and p == NP - 1),
                                        skip_group_check=True)

                        # one reciprocal for both pairs' denominators
                        dinv = dinvp.tile([C, NP], F32, tag="dinv",
                                          name=f"di_{rep}_{cb}")
                        nc.vector.reciprocal(dinv[:], pob[:, E:NP * PW:PW])
                        jo = cb % OSL
                        for p in range(NP):
                            nc.scalar.mul(
                                back["outs"][p][:, jo * E:(jo + 1) * E],
                                pob[:, p * PW:p * PW + E], dinv[:, p:p + 1])
                        if jo == OSL - 1:
                            # out DMAs go on the ACT HWDGE ring: the SP ring
                            # executes strictly FIFO, so an out DMA (gated on
                            # compute) queued there would head-of-line block
                            # the next iteration's input group loads
                            sb = cb // OSL
                            for p in range(NP):
                                r0 = (p * NOS + sb) * C
                                nc.scalar.dma_start(out[r0:r0 + C, :],
                                                    back["outs"][p][:])

    nc.compile()
    return nc


def _prepare_in_maps(q, k, q_rot, k_rot, v):
    b, h, n, d = q.shape
    e = v.shape[-1]
    nbh = b * h
    ht = np.float16
    qf = np.asarray(q).reshape(nbh, n, d).astype(ht)
    kf = np.asarray(k).reshape(nbh, n, d).astype(ht)
    qrf = np.asarray(q_rot).reshape(nbh, n, d).astype(ht)
    krf = np.asarray(k_rot).reshape(nbh, n, d).astype(ht)
    vf = np.asarray(v).reshape(nbh, n, e).astype(ht)
    mask2 = np.ascontiguousarray(
        np.tile(np.triu(np.ones((C, C), np.float32)), (1, 2)))

    in_maps = []
    for i in range(N_CORES):
        sel = [NP * i + p for p in range(NP)]
        in_all = np.zeros((NG * C, GW), ht)
        for p, s in enumerate(sel):
            for cseq in range(NCHUNK):
                g, j = divmod(cseq, CG)
                base = (p * CG + j) * CW
                rows = slice(g * C, (g + 1) * C)
                blk = slice(cseq * C, (cseq + 1) * C)
                in_all[rows, base + OFF_QT:base + OFF_QT + C] = qf[s][blk].T
                in_all[rows, base + OFF_KT:base + OFF_KT + C] = kf[s][blk].T
                in_all[rows, base + OFF_QRT:base + OFF_QRT + C] = qrf[s][blk].T
                in_all[rows, base + OFF_KRT:base + OFF_KRT + C] = krf[s][blk].T
                in_all[rows, base + OFF_KN:base + OFF_KN + D] = kf[s][blk]
                in_all[rows, base + OFF_KRN:base + OFF_KRN + D] = krf[s][blk]
                in_all[rows, base + OFF_V1:base + OFF_V1 + E] = vf[s][blk]
                in_all[rows, base + OFF_V1 + E] = 1.0
        in_maps.append(dict(in_all=in_all, mask2=mask2))
    return in_maps


def kernel(q, k, q_rot, k_rot, v, horizon=128, **run_kwargs):
    q = np.asarray(q)
    k = np.asarray(k)
    q_rot = np.asarray(q_rot)
    k_rot = np.asarray(k_rot)
    v = np.asarray(v)
    b, h, n, d = q.shape
    e = v.shape[-1]
    assert (b * h, n, d, e) == (N_CORES * NP, N, D, E), \
        "kernel is hardcoded for b*h=16, n=2048, d=128, e=64"

    if "nc" not in _cached:
        _cached["nc"] = build_kernel()
    nc = _cached["nc"]

    in_maps = _prepare_in_maps(q, k, q_rot, k_rot, v)
    res = run_bass_kernel_spmd(nc, in_maps, core_ids=list(range(N_CORES)),
                               **run_kwargs)

    outf = np.empty((b * h, n, e), dtype=np.float32)
    for i in range(N_CORES):
        o = res.results[i]["out"].reshape(NP, NOS, C, OSL, E)
        for p in range(NP):
            # [NOS, C, OSL, E] -> [NOS, OSL, C, E] -> [n, e]
            outf[NP * i + p] = (o[p].transpose(0, 2, 1, 3)
                                .reshape(n, e).astype(np.float32))
    if run_kwargs:
        kernel.last_results = res
    return outf.reshape(b, h, n, e)


if __name__ == "__main__":
    rng = np.random.default_rng(0)
    q = rng.random((2, 8, N, D), dtype=np.float32)
    k = rng.random((2, 8, N, D), dtype=np.float32)
    qr = rng.standard_normal((2, 8, N, D), dtype=np.float32)
    kr = rng.standard_normal((2, 8, N, D), dtype=np.float32)
    v = rng.random((2, 8, N, E), dtype=np.float32)
    o = kernel(q, k, qr, kr, v, 128)
    print("ok", o.shape, o.dtype, np.abs(o).mean())
